# revision 6
# baseline (speedup 1.0000x reference)
"""Trainium2 Bass kernel for nn_ConvEnhanced (conv/attn/quantum fused head).

Reference math per sample (x is (16,) f32, all in [0,1)):
    cls  = sigmoid(dot(x, w) + b)
    attn = mean_j sigmoid(a * x_j)
    q    = mean_j sin^2(pi * x_j / 2)        (the threshold/where is a no-op for x >= 0)
    out  = alpha * cls * attn + (1 - alpha) * q

Device strategy (pure data parallel over 8 cores, 524288 samples/core):
  - x shard lives in DRAM as (128, 65536) f32: partition p owns 4096 contiguous
    samples (16 contiguous floats each) -> full-bandwidth DMA.
  - ScalarE does the two transcendental passes:
        th = tanh((a/2) * x)        [sigmoid(ax) = 0.5 + 0.5*tanh(ax/2)]
        cs = sin(pi*x + pi/2)       [= cos(pi*x); sin^2(pi x/2) = (1-cos(pi x))/2]
    Tanh and Sin share one ACT table set (silu_and_others) -> single table load.
  - TensorE does the per-sample segmented sums (16 elems along the free dim)
    as 16 PSUM-accumulating matmuls per reduction with stride-16 moving APs:
        S_wx  += diag(w_j) @ x[:, j::16]     (fp32)
        S_th  += I @ th[:, j::16]            (fp16 in, fp32 accum)
        S_cs  += I @ cs[:, j::16]
  - ScalarE: t_c = tanh(0.5*S_wx + b/2)  ->  cls = 0.5*(1 + t_c)
  - VectorE tail combine:
        out = c0 + c1*t_c + c2*(S_th + t_c*S_th) + c3*S_cs
        c0 = alpha/4 + (1-alpha)/2, c1 = alpha/4, c2 = alpha/64, c3 = -(1-alpha)/32
"""

import numpy as np

try:
    import concourse.bass as bass  # noqa: F401
except ImportError:  # pragma: no cover
    import sys

    sys.path.insert(0, "/opt/trn_rl_repo")
    import concourse.bass as bass  # noqa: F401

B = 4_194_304  # total samples
N_CORES = 8
P = 128  # partitions
KE = 16  # elements per sample (4x4 patch)
B_LOC = B // N_CORES  # samples per core
SPP = B_LOC // P  # samples per partition (4096)

_NC_CACHE = {}


def _build(spp, t_tile):
    """Build the Bass/Tile program for one core (SPMD: identical on all cores).

    spp:    samples per partition held by this core
    t_tile: samples per partition processed per tile iteration
    """
    import concourse.bacc as bacc
    import concourse.bass as bass
    import concourse.tile as tile
    from concourse import mybir

    F32 = mybir.dt.float32
    F16 = mybir.dt.float16
    A = mybir.ActivationFunctionType
    Op = mybir.AluOpType

    assert spp % t_tile == 0
    n_tiles = spp // t_tile
    ft = KE * t_tile  # free elems per tile per partition

    nc = bacc.Bacc("TRN2", target_bir_lowering=False)
    x_d = nc.declare_dram_parameter("x", [P, spp * KE], F32, isOutput=False)
    wd_d = nc.declare_dram_parameter("wdiag", [P, KE * P], F32, isOutput=False)
    id_d = nc.declare_dram_parameter("ident", [P, P], F16, isOutput=False)
    c_d = nc.declare_dram_parameter("consts", [P, 8], F32, isOutput=False)
    o_d = nc.declare_dram_parameter("out", [P, spp], F32, isOutput=True)

    PI = float(np.pi)

    with tile.TileContext(nc) as tc:
        with (
            tc.tile_pool(name="const", bufs=1) as cpool,
            tc.tile_pool(name="xp", bufs=2) as xpool,
            tc.tile_pool(name="actp", bufs=2) as apool,
            tc.tile_pool(name="smallp", bufs=2) as spool,
            tc.tile_pool(name="psump", bufs=2, space="PSUM") as ppool,
        ):
            wd_sb = cpool.tile([P, KE * P], F32, tag="wd")
            nc.sync.dma_start(wd_sb[:], wd_d[:])
            id_sb = cpool.tile([P, P], F16, tag="id")
            nc.sync.dma_start(id_sb[:], id_d[:])
            c_sb = cpool.tile([P, 8], F32, tag="c")
            nc.sync.dma_start(c_sb[:], c_d[:])

            wd_v = wd_sb[:].rearrange("p (j m) -> p j m", j=KE)

            for t in range(n_tiles):
                x_t = xpool.tile([P, ft], F32, tag="x")
                nc.sync.dma_start(x_t[:], x_d[:, bass.ts(t, ft)])

                # th = tanh((a/2) x), cs = sin(pi x + pi/2) = cos(pi x)
                th_t = apool.tile([P, ft], F16, tag="th")
                nc.scalar.activation(th_t[:], x_t[:], A.Tanh, scale=c_sb[:, 0:1])
                cs_t = apool.tile([P, ft], F16, tag="cs")
                # cos(pi x) = sin(pi/2 - pi x); argument stays within [-pi, pi]
                nc.scalar.activation(
                    cs_t[:], x_t[:], A.Sin, bias=c_sb[:, 6:7], scale=-PI
                )

                ps_wx = ppool.tile([P, t_tile], F32, tag="pwx")
                ps_th = ppool.tile([P, t_tile], F32, tag="pth")
                ps_cs = ppool.tile([P, t_tile], F32, tag="pcs")

                x_v = x_t[:].rearrange("p (t j) -> p t j", j=KE)
                th_v = th_t[:].rearrange("p (t j) -> p t j", j=KE)
                cs_v = cs_t[:].rearrange("p (t j) -> p t j", j=KE)

                for j in range(KE):
                    nc.tensor.matmul(
                        ps_wx[:],
                        lhsT=wd_v[:, j, :],
                        rhs=x_v[:, :, j],
                        start=(j == 0),
                        stop=(j == KE - 1),
                    )
                for j in range(KE):
                    nc.tensor.matmul(
                        ps_th[:],
                        lhsT=id_sb[:],
                        rhs=th_v[:, :, j],
                        start=(j == 0),
                        stop=(j == KE - 1),
                    )
                for j in range(KE):
                    nc.tensor.matmul(
                        ps_cs[:],
                        lhsT=id_sb[:],
                        rhs=cs_v[:, :, j],
                        start=(j == 0),
                        stop=(j == KE - 1),
                    )

                # t_c = tanh(0.5*S_wx + b/2); cls = 0.5*(1+t_c)
                tc_t = spool.tile([P, t_tile], F32, tag="tc")
                nc.scalar.activation(
                    tc_t[:], ps_wx[:], A.Tanh, bias=c_sb[:, 1:2], scale=0.5
                )
                # m1 = t_c * S_th ; a1 = S_th + m1
                m1 = spool.tile([P, t_tile], F32, tag="m1")
                nc.vector.tensor_mul(m1[:], tc_t[:], ps_th[:])
                a1 = spool.tile([P, t_tile], F32, tag="a1")
                nc.vector.tensor_add(a1[:], m1[:], ps_th[:])
                # t1 = c1*t_c + c0
                t1 = spool.tile([P, t_tile], F32, tag="t1")
                nc.vector.tensor_scalar(
                    t1[:], tc_t[:], c_sb[:, 2:3], c_sb[:, 3:4], Op.mult, Op.add
                )
                # p1 = c3*S_cs + t1
                p1 = spool.tile([P, t_tile], F32, tag="p1")
                nc.vector.scalar_tensor_tensor(
                    p1[:], ps_cs[:], c_sb[:, 5:6], t1[:], Op.mult, Op.add
                )
                # out = c2*a1 + p1
                o_t = spool.tile([P, t_tile], F32, tag="o")
                nc.vector.scalar_tensor_tensor(
                    o_t[:], a1[:], c_sb[:, 4:5], p1[:], Op.mult, Op.add
                )
                nc.sync.dma_start(o_d[:, bass.ts(t, t_tile)], o_t[:])

    nc.compile()
    return nc


def get_nc(spp=SPP, t_tile=512):
    key = (spp, t_tile)
    if key not in _NC_CACHE:
        _NC_CACHE[key] = _build(spp, t_tile)
    return _NC_CACHE[key]


def make_const_inputs(conv_w, conv_b, attn_w, alpha):
    """Host-side packing of the tiny runtime parameters into device tensors."""
    w = np.asarray(conv_w, dtype=np.float32).reshape(KE)
    b = float(np.asarray(conv_b, dtype=np.float32).reshape(-1)[0])
    a = float(np.asarray(attn_w, dtype=np.float32).reshape(-1)[0])
    al = float(np.asarray(alpha, dtype=np.float32))

    wdiag = np.zeros((P, KE, P), dtype=np.float32)
    idx = np.arange(P)
    wdiag[idx, :, idx] = w[None, :]
    wdiag = np.ascontiguousarray(wdiag.reshape(P, KE * P))

    ident = np.ascontiguousarray(np.eye(P, dtype=np.float16))

    row = np.zeros(8, dtype=np.float32)
    row[0] = a / 2.0  # scale for tanh(a x / 2)
    row[1] = b / 2.0  # bias for tanh(0.5 S_wx + b/2)
    row[2] = al / 4.0  # c1
    row[3] = al / 4.0 + (1.0 - al) / 2.0  # c0
    row[4] = al / 64.0  # c2
    row[5] = -(1.0 - al) / 32.0  # c3
    row[6] = np.pi / 2.0  # bias for sin(pi x + pi/2) = cos(pi x)
    consts = np.ascontiguousarray(np.tile(row[None, :], (P, 1)))
    return wdiag, ident, consts


def kernel(x, conv_w, conv_b, attn_w, alpha):
    from concourse.bass_utils import run_bass_kernel_spmd

    x = np.ascontiguousarray(np.asarray(x, dtype=np.float32))
    assert x.shape == (B, 1, KE // 4, KE // 4) or x.size == B * KE
    xs = x.reshape(N_CORES, P, SPP * KE)

    wdiag, ident, consts = make_const_inputs(conv_w, conv_b, attn_w, alpha)

    nc = get_nc()
    in_maps = [
        {"x": xs[c], "wdiag": wdiag, "ident": ident, "consts": consts}
        for c in range(N_CORES)
    ]
    res = run_bass_kernel_spmd(nc, in_maps, list(range(N_CORES)))
    out = np.concatenate(
        [np.asarray(res.results[c]["out"], dtype=np.float32).reshape(-1) for c in range(N_CORES)]
    )
    return out
